# revision 2
# baseline (speedup 1.0000x reference)
"""Trainium2 Bass kernel for nn_DualLSTM: dual-LSTM scan + 2-layer FC head.

Strategy (8 NeuronCores, SPMD, no collectives):
  - The sequential scan is replaced by NSWEEP Picard (fixed-point) sweeps.
    Each sweep recomputes all T gates DENSELY (PE at N=TB efficiency) from
    the previous sweep's h estimates, then solves the c-recurrence exactly
    with the DVE tensor_tensor_scan instruction (c_t = (f_e*f_c)_t*c_{t-1}
    + U_t is linear given gates), then forms h. Converges ~0.42x/sweep;
    10 sweeps reach ~2e-4 end-to-end (threshold 2e-2).
  - Branch algebra (m = mask):
      P_x = sig(i_x)*tanh(g_x);  U = m ? f_c*P_e + P_c : f_e*P_c + P_e
      c_t = (f_e*f_c)*c_{t-1} + U
      X_x = f_x*c_{t-1} + P_x;  SEL_e = m ? X_e : c;  SEL_c = m ? c : X_c
      h_x = sig(o_x) * tanh(SEL_x);  outs = h_e + h_c
  - A-stage (once): input-side gate terms A = mask-sel(x_t) @ Wih_perm.T + b
    as batched matmuls -> DRAM f16, streamed back each sweep and injected
    into PSUM via an identity matmul (start=True of each accumulation).
  - fc1 (replicated): hmidT = relu(fc1_W @ outs.T + b1) -> DRAM f16.
  - fc2 (V-sharded 1250 rows/core): outT_shard = fc2_Ws @ hmid + b2 -> f32.
  - Host: concat shards, transpose -> [2048, 10000] f32.

Gate-column layout (32 cols, each col = 128 gate rows):
  0-7 i (en 0-3, cn 4-7) | 8-15 f | 16-23 g | 24-31 o
  sigmoid on i,f,o; tanh on g.
"""

import os
import numpy as np
from contextlib import ExitStack

import concourse.bass as bass
import concourse.bacc as bacc
import concourse.mybir as mybir
import concourse.tile as tile
from concourse.ap import AP
from concourse.bass_utils import run_bass_kernel_spmd

F16 = mybir.dt.float16
F32 = mybir.dt.float32
AF = mybir.ActivationFunctionType
OP = mybir.AluOpType

T, V, E, H = 2048, 10000, 256, 512
NCORES = 8
NCOLS = 32            # combined gate columns (2 cells x 16)
KC = H // 128         # 4 K-chunks for recurrent weights
EK = 3                # K-chunks for A-stage (E=256 + bias col + pad -> 384)
VP = 10112            # V padded to 79*128 for fc1/hmid
MT1 = VP // 128       # 79 fc1 M-tiles
VSH = V // NCORES     # 1250 fc2 rows per core
VSP = 1280            # padded shard
MT2 = VSP // 128      # 10 fc2 M-tiles
TB = 256              # sweep t-block
NTB = T // TB         # 8
NSWEEP = 10
TP1 = T + 1           # Hp cols per chunk (col 0 = h_{-1} = 0)
CS = TB + 1           # cS cols per chunk (col 0 = c_{t-1} carry-in)

# col blocks of 4: [i_en, i_cn, f_en, f_cn, g_en, g_cn, o_en, o_cn]
EN_COLS = frozenset(c for c in range(NCOLS) if (c // 4) % 2 == 0)


def _strided(t_ap: AP, off: int, stride: int, n: int, width: int) -> AP:
    """[128, n, width] view of a [128, N] tile: block j at col off+j*stride."""
    pstep, pcount = t_ap.ap[0]
    return AP(t_ap.tensor, t_ap.offset + off,
              [[pstep, pcount], [stride, n], [1, width]])


def _v4(t_ap: AP) -> AP:
    """[128, 4*TB] -> [128, 4, TB]."""
    return t_ap.rearrange("p (k x) -> p k x", k=4)


def build_program():
    nc = bacc.Bacc("TRN2", target_bir_lowering=False, debug=False,
                   num_devices=NCORES)

    # ---- DRAM I/O ----
    wsb_d = nc.dram_tensor("wsb", [128, NCOLS * KC * 128], F16, kind="ExternalInput")
    wih_d = nc.dram_tensor("wih", [128, NCOLS * EK * 128], F16, kind="ExternalInput")
    xte_d = nc.dram_tensor("xte", [128, EK * T], F16, kind="ExternalInput")
    xtc_d = nc.dram_tensor("xtc", [128, EK * T], F16, kind="ExternalInput")
    ident_d = nc.dram_tensor("ident", [128, 128], F16, kind="ExternalInput")
    mT_d = nc.dram_tensor("mT", [128, T], F16, kind="ExternalInput")
    w1t_d = nc.dram_tensor("w1t", [H, VP], F16, kind="ExternalInput")
    b1c_d = nc.dram_tensor("b1c", [128, MT1], F32, kind="ExternalInput")
    w2t_d = nc.dram_tensor("w2t", [VP, VSP], F16, kind="ExternalInput")
    b2c_d = nc.dram_tensor("b2c", [128, MT2], F32, kind="ExternalInput")
    outT_d = nc.dram_tensor("outT", [VSP, T], F32, kind="ExternalOutput")

    # internal DRAM scratch
    a2_d = nc.dram_tensor("a2_scratch", [NCOLS, 128, T], F16)
    hmid_d = nc.dram_tensor("hmid_scratch", [VP, T], F16)

    with tile.TileContext(nc) as tc, ExitStack() as stk:
        # ---------- persistent SBUF ----------
        pers = stk.enter_context(tc.tile_pool(name="pers", bufs=1))
        outsT = pers.tile([128, 4 * T], F16, tag="outsT")   # col = k*T + t
        b1c = pers.tile([128, MT1], F32, tag="b1c")
        b2c = pers.tile([128, MT2], F32, tag="b2c")
        ident = pers.tile([128, 128], F16, tag="ident")
        mTs = pers.tile([128, T], F16, tag="mTs")
        nc.sync.dma_start(b1c[:], b1c_d[:])
        nc.sync.dma_start(b2c[:], b2c_d[:])
        nc.sync.dma_start(ident[:], ident_d[:])
        nc.sync.dma_start(mTs[:], mT_d[:])

        with tc.tile_pool(name="scanp", bufs=1) as scanp:
            wsb = scanp.tile([128, NCOLS * KC * 128], F16, tag="wsb")
            nc.sync.dma_start(wsb[:], wsb_d[:])
            # Hp[parity]: [p, cell-chunk k 0..7, TP1]; col t holds h_{t-1}
            Hp = [scanp.tile([128, 8 * TP1], F16, tag=f"Hp{i}",
                             name=f"Hp{i}") for i in range(2)]
            nc.gpsimd.memset(Hp[0][:], 0.0)
            nc.gpsimd.memset(Hp[1][:], 0.0)

            # ---------- A-stage ----------
            with tc.tile_pool(name="astg", bufs=1) as ast_pool, \
                 tc.tile_pool(name="apsum", bufs=4, space="PSUM") as aps_pool, \
                 tc.tile_pool(name="acopy", bufs=4) as acp_pool:
                wih = ast_pool.tile([128, NCOLS * EK * 128], F16, tag="wih")
                xte = ast_pool.tile([128, EK * T], F16, tag="xte")
                xtc = ast_pool.tile([128, EK * T], F16, tag="xtc")
                nc.sync.dma_start(wih[:], wih_d[:])
                nc.sync.dma_start(xte[:], xte_d[:])
                nc.sync.dma_start(xtc[:], xtc_d[:])
                for tb in range(4):  # t-blocks of 512
                    for c in range(NCOLS):
                        xts = xte if c in EN_COLS else xtc
                        ps = aps_pool.tile([128, 512], F32, tag="aps")
                        for kx in range(EK):
                            nc.tensor.matmul(
                                ps[:],
                                wih[:, (c * EK + kx) * 128:
                                    (c * EK + kx + 1) * 128],
                                xts[:, kx * T + tb * 512:
                                    kx * T + tb * 512 + 512],
                                start=(kx == 0), stop=(kx == EK - 1))
                        stg = acp_pool.tile([128, 512], F16, tag="astg")
                        nc.vector.tensor_copy(stg[:], ps[:])
                        nc.sync.dma_start(
                            a2_d[c][:, tb * 512:(tb + 1) * 512], stg[:])

            # ---------- Picard sweeps ----------
            with tc.tile_pool(name="aslp", bufs=6) as aslp, \
                 tc.tile_pool(name="gps", bufs=4, space="PSUM") as gps, \
                 tc.tile_pool(name="sgp", bufs=2) as sgp, \
                 tc.tile_pool(name="wkp", bufs=2) as wkp, \
                 tc.tile_pool(name="csp", bufs=2) as csp:
                for s in range(NSWEEP):
                    cur, nxt = Hp[s % 2], Hp[(s + 1) % 2]
                    last = (s == NSWEEP - 1)
                    cs_prev = None
                    for tb in range(NTB):
                        t0 = tb * TB
                        # S tiles: [grp][cell] -> [128, KC*TB]
                        S = [[sgp.tile([128, KC * TB], F16,
                                       tag=f"S{g}{ce}", name=f"S{g}{ce}")
                              for ce in range(2)] for g in range(4)]
                        # ---- gates: psum = A + W*hprev; sigma/tanh ----
                        for c in range(NCOLS):
                            hoff = 0 if c in EN_COLS else 4
                            asl = aslp.tile([128, TB], F16, tag="asl")
                            nc.sync.dma_start(
                                asl[:], a2_d[c][:, t0:t0 + TB])
                            ps = gps.tile([128, TB], F32, tag="gps")
                            nc.tensor.matmul(ps[:], ident[:], asl[:],
                                             start=True, stop=(s == 0))
                            if s > 0:
                                for k in range(KC):
                                    nc.tensor.matmul(
                                        ps[:],
                                        wsb[:, (c * KC + k) * 128:
                                            (c * KC + k + 1) * 128],
                                        cur[:, (hoff + k) * TP1 + t0:
                                            (hoff + k) * TP1 + t0 + TB],
                                        start=False, stop=(k == KC - 1))
                            grp, cell, chunk = c // 8, (c // 4) % 2, c % 4
                            nc.scalar.activation(
                                S[grp][cell][:, chunk * TB:(chunk + 1) * TB],
                                ps[:], AF.Tanh if grp == 2 else AF.Sigmoid)
                        Si_e, Si_c = S[0]
                        Sf_e, Sf_c = S[1]
                        Tg_e, Tg_c = S[2]
                        So_e, So_c = S[3]
                        # ---- DVE block ----
                        wt = lambda tg: wkp.tile([128, KC * TB], F16,
                                                 tag=tg, name=tg)
                        mk4 = _strided(mTs[:], t0, 0, 4, TB)
                        P_e, P_c = wt("P_e"), wt("P_c")
                        t1, t2, t3, t4 = wt("t1"), wt("t2"), wt("t3"), wt("t4")
                        selE, selC = wt("selE"), wt("selC")
                        tsE, tsC = wt("tsE"), wt("tsC")
                        nc.vector.tensor_tensor(P_e[:], Si_e[:], Tg_e[:],
                                                OP.mult)
                        nc.vector.tensor_tensor(P_c[:], Si_c[:], Tg_c[:],
                                                OP.mult)
                        # t1 = Pe1 = P_e*m ; t2 = Pe0 = P_e - Pe1
                        nc.vector.tensor_tensor(_v4(t1[:]), _v4(P_e[:]),
                                                mk4, OP.mult)
                        nc.vector.tensor_tensor(t2[:], P_e[:], t1[:],
                                                OP.subtract)
                        # t3 = Pc1 ; t4 = Pc0
                        nc.vector.tensor_tensor(_v4(t3[:]), _v4(P_c[:]),
                                                mk4, OP.mult)
                        nc.vector.tensor_tensor(t4[:], P_c[:], t3[:],
                                                OP.subtract)
                        # FF -> Tg_e (tanh-g free after P)
                        nc.vector.tensor_tensor(Tg_e[:], Sf_e[:], Sf_c[:],
                                                OP.mult)
                        # U = f_c*Pe1 + Pc1 + f_e*Pc0 + Pe0 -> t1
                        nc.vector.tensor_tensor(t1[:], Sf_c[:], t1[:],
                                                OP.mult)
                        nc.vector.tensor_tensor(t4[:], Sf_e[:], t4[:],
                                                OP.mult)
                        nc.vector.tensor_tensor(t1[:], t1[:], t4[:], OP.add)
                        nc.vector.tensor_tensor(t2[:], t2[:], t3[:], OP.add)
                        nc.vector.tensor_tensor(t1[:], t1[:], t2[:], OP.add)
                        # ---- c scan ----
                        cS = csp.tile([128, KC * CS], F16, tag="cS",
                                      name="cS")
                        if tb == 0:
                            nc.gpsimd.memset(
                                _strided(cS[:], 0, CS, 4, 1), 0.0)
                        else:
                            nc.vector.tensor_copy(
                                _strided(cS[:], 0, CS, 4, 1),
                                _strided(cs_prev[:], TB, CS, 4, 1))
                        for j in range(KC):
                            ini = 0.0 if tb == 0 else \
                                cs_prev[:, j * CS + TB:j * CS + TB + 1]
                            nc.vector.tensor_tensor_scan(
                                cS[:, j * CS + 1:j * CS + 1 + TB],
                                Tg_e[:, j * TB:(j + 1) * TB],
                                t1[:, j * TB:(j + 1) * TB],
                                ini, OP.mult, OP.add)
                        cprev = _strided(cS[:], 0, CS, 4, TB)
                        c4 = _strided(cS[:], 1, CS, 4, TB)
                        # X_e -> t2 ; X_c -> t3
                        nc.vector.tensor_tensor(_v4(t2[:]), _v4(Sf_e[:]),
                                                cprev, OP.mult)
                        nc.vector.tensor_tensor(t2[:], t2[:], P_e[:], OP.add)
                        nc.vector.tensor_tensor(_v4(t3[:]), _v4(Sf_c[:]),
                                                cprev, OP.mult)
                        nc.vector.tensor_tensor(t3[:], t3[:], P_c[:], OP.add)
                        # SEL_e = c + m*(X_e - c)
                        nc.vector.tensor_tensor(_v4(t4[:]), _v4(t2[:]),
                                                c4, OP.subtract)
                        nc.vector.tensor_tensor(_v4(t4[:]), _v4(t4[:]),
                                                mk4, OP.mult)
                        nc.vector.tensor_tensor(_v4(selE[:]), _v4(t4[:]),
                                                c4, OP.add)
                        # SEL_c = X_c - m*(X_c - c)
                        nc.vector.tensor_tensor(_v4(t4[:]), _v4(t3[:]),
                                                c4, OP.subtract)
                        nc.vector.tensor_tensor(_v4(t4[:]), _v4(t4[:]),
                                                mk4, OP.mult)
                        nc.vector.tensor_tensor(selC[:], t3[:], t4[:],
                                                OP.subtract)
                        nc.scalar.activation(tsE[:], selE[:], AF.Tanh)
                        nc.scalar.activation(tsC[:], selC[:], AF.Tanh)
                        # h -> Hp[nxt] shifted by +1 col
                        hpE = _strided(nxt[:], 0 * TP1 + t0 + 1, TP1, 4, TB)
                        hpC = _strided(nxt[:], 4 * TP1 + t0 + 1, TP1, 4, TB)
                        nc.vector.tensor_tensor(hpE, _v4(So_e[:]),
                                                _v4(tsE[:]), OP.mult)
                        nc.vector.tensor_tensor(hpC, _v4(So_c[:]),
                                                _v4(tsC[:]), OP.mult)
                        if last:
                            ovT = _strided(outsT[:], t0, T, 4, TB)
                            nc.vector.tensor_tensor(ovT, hpE, hpC, OP.add)
                        cs_prev = cS

        # ---------- fc1 ----------
        with tc.tile_pool(name="f1w", bufs=1) as f1w, \
             tc.tile_pool(name="f1ps", bufs=4, space="PSUM") as f1ps, \
             tc.tile_pool(name="f1st", bufs=4) as f1st:
            w1sb = f1w.tile([128, 4 * VP], F16, tag="w1sb")
            for k in range(4):
                nc.sync.dma_start(w1sb[:, k * VP:(k + 1) * VP],
                                  w1t_d[k * 128:(k + 1) * 128, :])
            for nb in range(4):
                for m in range(MT1):
                    ps = f1ps.tile([128, 512], F32, tag="f1p")
                    for k in range(4):
                        nc.tensor.matmul(
                            ps[:],
                            w1sb[:, k * VP + m * 128: k * VP + m * 128 + 128],
                            outsT[:, k * T + nb * 512: k * T + nb * 512 + 512],
                            start=(k == 0), stop=(k == 3))
                    hst = f1st.tile([128, 512], F16, tag="f1h")
                    nc.scalar.activation(hst[:], ps[:], AF.Relu,
                                         bias=b1c[:, m:m + 1])
                    nc.sync.dma_start(
                        hmid_d[m * 128:(m + 1) * 128, nb * 512:(nb + 1) * 512],
                        hst[:])

        # ---------- fc2 ----------
        MGROUPS = [(0, 4), (4, 4), (8, 2)]
        with tc.tile_pool(name="hblk", bufs=MT1) as hbp, \
             tc.tile_pool(name="w2p", bufs=6) as w2p, \
             tc.tile_pool(name="f2ps", bufs=5, space="PSUM") as f2ps, \
             tc.tile_pool(name="f2st", bufs=4) as f2st:
            for nb in range(4):
                hts = []
                for k2 in range(MT1):
                    ht = hbp.tile([128, 512], F16, tag="hblk")
                    nc.sync.dma_start(
                        ht[:], hmid_d[k2 * 128:(k2 + 1) * 128,
                                      nb * 512:(nb + 1) * 512])
                    hts.append(ht)
                for (m0, mw) in MGROUPS:
                    pss = [f2ps.tile([128, 512], F32, tag="f2p", name="f2p")
                           for _ in range(mw)]
                    for k2 in range(MT1):
                        w2c = w2p.tile([128, 512], F16, tag="w2c")
                        nc.sync.dma_start(
                            w2c[:, 0:mw * 128],
                            w2t_d[k2 * 128:(k2 + 1) * 128,
                                  m0 * 128: m0 * 128 + mw * 128])
                        for mi in range(mw):
                            nc.tensor.matmul(
                                pss[mi][:], w2c[:, mi * 128:(mi + 1) * 128],
                                hts[k2][:],
                                start=(k2 == 0), stop=(k2 == MT1 - 1))
                    for mi in range(mw):
                        m = m0 + mi
                        ost = f2st.tile([128, 512], F32, tag="f2o")
                        nc.scalar.activation(ost[:], pss[mi][:], AF.Identity,
                                             bias=b2c[:, m:m + 1])
                        nc.sync.dma_start(
                            outT_d[m * 128:(m + 1) * 128,
                                   nb * 512:(nb + 1) * 512], ost[:])

    nc.compile()
    return nc


# ---------------- host side ----------------

_NC_CACHE = {}


def _get_program():
    if "nc" not in _NC_CACHE:
        _NC_CACHE["nc"] = build_program()
    return _NC_CACHE["nc"]


def _build_big(Wen, Wcn):
    """Stack two cells' torch-gate-order rows [i,f,g,o] into combined
    [i_en, i_cn, f_en, f_cn, g_en, g_cn, o_en, o_cn] order."""
    blocks = []
    for gi in range(4):
        blocks.append(Wen[gi * H:(gi + 1) * H])
        blocks.append(Wcn[gi * H:(gi + 1) * H])
    return np.concatenate(blocks, axis=0)


def _pack_lhsT(bigw, nk):
    """[4096, nk*128] -> [128, 32*nk*128] with tile (c,k) at col
    (c*nk+k)*128 + m, element [p] = bigw[c*128+m, k*128+p]."""
    arr = bigw.reshape(NCOLS, 128, nk, 128)           # [c, m, k, p]
    return np.ascontiguousarray(arr.transpose(3, 0, 2, 1)
                                ).reshape(128, NCOLS * nk * 128)


def host_prep(inputs):
    tok = np.asarray(inputs["token_ids"]).astype(np.int64)
    msk = np.asarray(inputs["mask"]).astype(np.float32)
    emb = np.asarray(inputs["emb"], dtype=np.float32)
    f32 = lambda n: np.asarray(inputs[n], dtype=np.float32)
    Wih_en, Whh_en = f32("Wih_en"), f32("Whh_en")
    bih_en, bhh_en = f32("bih_en"), f32("bhh_en")
    Wih_cn, Whh_cn = f32("Wih_cn"), f32("Whh_cn")
    bih_cn, bhh_cn = f32("bih_cn"), f32("bhh_cn")
    fc1_W, fc1_b = f32("fc1_W"), f32("fc1_b")
    fc2_W, fc2_b = f32("fc2_W"), f32("fc2_b")

    # --- recurrent weights ---
    bigwhh = _build_big(Whh_en, Whh_cn)               # [4096, 512]
    wsb = _pack_lhsT(bigwhh, KC).astype(np.float16)

    # --- A-stage weights: [Wih | b | 0] augmented to K=384 ---
    def aug(Wih, b):
        return np.concatenate(
            [Wih, b[:, None],
             np.zeros((4 * H, EK * 128 - E - 1), np.float32)], axis=1)
    ae = aug(Wih_en, bih_en + bhh_en)                 # [2048, 384]
    ac = aug(Wih_cn, bih_cn + bhh_cn)
    bigwih = _build_big(ae, ac)                       # [4096, 384]
    wih = _pack_lhsT(bigwih, EK).astype(np.float16)

    # --- X augmented, mask-folded, transposed ---
    X = emb[tok]                                      # [T, E]
    ones = np.ones((T, 1), np.float32)
    zpad = np.zeros((T, EK * 128 - E - 1), np.float32)
    xa_en = np.concatenate([X * msk[:, None], ones, zpad], axis=1)
    xa_cn = np.concatenate([X * (1.0 - msk)[:, None], ones, zpad], axis=1)
    xte = np.ascontiguousarray(
        xa_en.reshape(T, EK, 128).transpose(2, 1, 0)).reshape(128, EK * T)
    xtc = np.ascontiguousarray(
        xa_cn.reshape(T, EK, 128).transpose(2, 1, 0)).reshape(128, EK * T)

    mT = np.ascontiguousarray(
        np.broadcast_to(msk[None, :], (128, T))).astype(np.float16)
    ident = np.eye(128, dtype=np.float16)

    # --- fc1 ---
    w1p = np.zeros((VP, H), np.float32)
    w1p[:V] = fc1_W
    w1t = np.ascontiguousarray(w1p.T).astype(np.float16)   # [512, VP]
    b1p = np.zeros((VP,), np.float32)
    b1p[:V] = fc1_b
    b1c = np.ascontiguousarray(b1p.reshape(MT1, 128).T)    # [128, MT1]

    # --- fc2 shards ---
    shard_w, shard_b = [], []
    for s in range(NCORES):
        w2p_ = np.zeros((VSP, VP), np.float32)
        w2p_[:VSH, :V] = fc2_W[s * VSH:(s + 1) * VSH]
        shard_w.append(np.ascontiguousarray(w2p_.T).astype(np.float16))
        b2p = np.zeros((VSP,), np.float32)
        b2p[:VSH] = fc2_b[s * VSH:(s + 1) * VSH]
        shard_b.append(np.ascontiguousarray(b2p.reshape(MT2, 128).T))

    common = dict(wsb=wsb, wih=wih, xte=xte.astype(np.float16),
                  xtc=xtc.astype(np.float16), mT=mT, ident=ident,
                  w1t=w1t, b1c=b1c)
    in_maps = []
    for s in range(NCORES):
        m = dict(common)
        m["w2t"] = shard_w[s]
        m["b2c"] = shard_b[s]
        in_maps.append(m)
    return in_maps


LAST_RESULT = None


def _install_ntff_shim():
    """The agent image lacks antenv.axon_hooks; register the ctypes NTFF
    profiling hook manually so trace=True works."""
    import sys, types
    if "antenv.axon_hooks" in sys.modules:
        return
    import antenv
    mod = types.ModuleType("antenv.axon_hooks")
    _h = [None]
    mod.set_axon_ntff_profile_hook = lambda h: _h.__setitem__(0, h)
    mod.get_axon_ntff_profile_hook = lambda: _h[0]
    sys.modules["antenv.axon_hooks"] = mod
    antenv.axon_hooks = mod
    from trn_agent_boot.trn_boot import _ntff_profile_via_ctypes
    mod.set_axon_ntff_profile_hook(
        _ntff_profile_via_ctypes("/opt/axon/libaxon_pjrt.so"))


def kernel(**inputs):
    global LAST_RESULT
    trace = bool(os.environ.get("DUALLSTM_TRACE"))
    if trace:
        _install_ntff_shim()
    nc = _get_program()
    in_maps = host_prep(inputs)
    res = run_bass_kernel_spmd(nc, in_maps, core_ids=list(range(NCORES)),
                               trace=trace)
    LAST_RESULT = res
    out = np.empty((T, V), np.float32)
    for s in range(NCORES):
        out[:, s * VSH:(s + 1) * VSH] = res.results[s]["outT"][:VSH].T
    return out


# revision 6
# speedup vs baseline: 1.4208x; 1.4208x over previous
"""Trainium2 Bass kernel for nn_DualLSTM: dual-LSTM scan + 2-layer FC head.

Strategy (8 NeuronCores, SPMD, no collectives):
  - The sequential scan is replaced by NSWEEP Picard (fixed-point) sweeps.
    Each sweep recomputes all T gates DENSELY (PE at N=TB efficiency) from
    the previous sweep's h estimates, then solves the c-recurrence exactly
    with the DVE tensor_tensor_scan instruction (c_t = (f_e*f_c)_t*c_{t-1}
    + U_t is linear given gates), then forms h. Converges ~0.42x/sweep;
    10 sweeps reach ~2e-4 end-to-end (threshold 2e-2).
  - Branch algebra (m = mask):
      P_x = sig(i_x)*tanh(g_x);  U = m ? f_c*P_e + P_c : f_e*P_c + P_e
      c_t = (f_e*f_c)*c_{t-1} + U
      X_x = f_x*c_{t-1} + P_x;  SEL_e = m ? X_e : c;  SEL_c = m ? c : X_c
      h_x = sig(o_x) * tanh(SEL_x);  outs = h_e + h_c
  - A-stage (once): input-side gate terms A = mask-sel(x_t) @ Wih_perm.T + b
    as batched matmuls -> DRAM f16, streamed back each sweep and injected
    into PSUM via an identity matmul (start=True of each accumulation).
  - fc1 (replicated): hmidT = relu(fc1_W @ outs.T + b1) -> DRAM f16.
  - fc2 (V-sharded 1250 rows/core): outT_shard = fc2_Ws @ hmid + b2 -> f32.
  - Host: concat shards, transpose -> [2048, 10000] f32.

Gate-column layout (32 cols, each col = 128 gate rows):
  0-7 i (en 0-3, cn 4-7) | 8-15 f | 16-23 g | 24-31 o
  sigmoid on i,f,o; tanh on g.
"""

import os
import numpy as np
from contextlib import ExitStack

import concourse.bass as bass
import concourse.bacc as bacc
import concourse.mybir as mybir
import concourse.tile as tile
from concourse.ap import AP
from concourse.bass_utils import run_bass_kernel_spmd

F16 = mybir.dt.float16
F32 = mybir.dt.float32
AF = mybir.ActivationFunctionType
OP = mybir.AluOpType

T, V, E, H = 2048, 10000, 256, 512
NCORES = 8
NCOLS = 32            # combined gate columns (2 cells x 16)
KC = H // 128         # 4 K-chunks for recurrent weights
EK = 3                # K-chunks for A-stage (E=256 + bias col + pad -> 384)
VP = 10112            # V padded to 79*128 for fc1/hmid
MT1 = VP // 128       # 79 fc1 M-tiles
VSH = V // NCORES     # 1250 fc2 rows per core
VSP = 1280            # padded shard
MT2 = VSP // 128      # 10 fc2 M-tiles
TB = 256              # sweep t-block
NTB = T // TB         # 8
NSWEEP = 8
TP1 = T + 1           # Hp cols per chunk (col 0 = h_{-1} = 0)
CS = TB + 1           # cS cols per chunk (col 0 = c_{t-1} carry-in)

# col blocks of 4: [i_en, i_cn, f_en, f_cn, g_en, g_cn, o_en, o_cn]
EN_COLS = frozenset(c for c in range(NCOLS) if (c // 4) % 2 == 0)


def _strided(t_ap: AP, off: int, stride: int, n: int, width: int) -> AP:
    """[128, n, width] view of a [128, N] tile: block j at col off+j*stride."""
    pstep, pcount = t_ap.ap[0]
    return AP(t_ap.tensor, t_ap.offset + off,
              [[pstep, pcount], [stride, n], [1, width]])


def _v4(t_ap: AP) -> AP:
    """[128, 4*TB] -> [128, 4, TB]."""
    return t_ap.rearrange("p (k x) -> p k x", k=4)


def build_program():
    nc = bacc.Bacc("TRN2", target_bir_lowering=False, debug=False,
                   num_devices=NCORES)

    # ---- DRAM I/O ----
    wsb_d = nc.dram_tensor("wsb", [128, NCOLS * KC * 128], F16, kind="ExternalInput")
    wih_d = nc.dram_tensor("wih", [128, NCOLS * EK * 128], F16, kind="ExternalInput")
    xte_d = nc.dram_tensor("xte", [128, EK * T], F16, kind="ExternalInput")
    xtc_d = nc.dram_tensor("xtc", [128, EK * T], F16, kind="ExternalInput")
    ident_d = nc.dram_tensor("ident", [128, 128], F16, kind="ExternalInput")
    mT_d = nc.dram_tensor("mT", [128, T], F16, kind="ExternalInput")
    w1t_d = nc.dram_tensor("w1t", [H, VP], F16, kind="ExternalInput")
    b1c_d = nc.dram_tensor("b1c", [128, MT1], F32, kind="ExternalInput")
    w2t_d = nc.dram_tensor("w2t", [VP, VSP], F16, kind="ExternalInput")
    b2c_d = nc.dram_tensor("b2c", [128, MT2], F32, kind="ExternalInput")
    outT_d = nc.dram_tensor("outT", [VSP, T], F32, kind="ExternalOutput")

    # internal DRAM scratch
    a2_d = nc.dram_tensor("a2_scratch", [NCOLS, 128, T], F16)
    hmid_d = nc.dram_tensor("hmid_scratch", [VP, T], F16)

    with tile.TileContext(nc) as tc, ExitStack() as stk:
        # ---------- persistent SBUF ----------
        pers = stk.enter_context(tc.tile_pool(name="pers", bufs=1))
        outsT = pers.tile([128, 4 * T], F16, tag="outsT")   # col = k*T + t
        b1c = pers.tile([128, MT1], F32, tag="b1c")
        b2c = pers.tile([128, MT2], F32, tag="b2c")
        ident = pers.tile([128, 128], F16, tag="ident")
        mTs = pers.tile([128, T], F16, tag="mTs")
        nc.sync.dma_start(b1c[:], b1c_d[:])
        nc.sync.dma_start(b2c[:], b2c_d[:])
        nc.sync.dma_start(ident[:], ident_d[:])
        nc.sync.dma_start(mTs[:], mT_d[:])

        with tc.tile_pool(name="scanp", bufs=1) as scanp:
            wsb = scanp.tile([128, NCOLS * KC * 128], F16, tag="wsb")
            nc.sync.dma_start(wsb[:], wsb_d[:])
            # Hp[parity]: [p, cell-chunk k 0..7, TP1]; col t holds h_{t-1}
            Hp = [scanp.tile([128, 8 * TP1], F16, tag=f"Hp{i}",
                             name=f"Hp{i}") for i in range(2)]
            nc.gpsimd.memset(Hp[0][:], 0.0)
            nc.gpsimd.memset(Hp[1][:], 0.0)

            # ---------- A-stage ----------
            with tc.tile_pool(name="astg", bufs=1) as ast_pool, \
                 tc.tile_pool(name="apsum", bufs=4, space="PSUM") as aps_pool, \
                 tc.tile_pool(name="acopy", bufs=4) as acp_pool:
                wih = ast_pool.tile([128, NCOLS * EK * 128], F16, tag="wih")
                xte = ast_pool.tile([128, EK * T], F16, tag="xte")
                xtc = ast_pool.tile([128, EK * T], F16, tag="xtc")
                nc.sync.dma_start(wih[:], wih_d[:])
                nc.sync.dma_start(xte[:], xte_d[:])
                nc.sync.dma_start(xtc[:], xtc_d[:])
                for tb in range(4):  # t-blocks of 512
                    for c in range(NCOLS):
                        xts = xte if c in EN_COLS else xtc
                        ps = aps_pool.tile([128, 512], F32, tag="aps")
                        for kx in range(EK):
                            nc.tensor.matmul(
                                ps[:],
                                wih[:, (c * EK + kx) * 128:
                                    (c * EK + kx + 1) * 128],
                                xts[:, kx * T + tb * 512:
                                    kx * T + tb * 512 + 512],
                                start=(kx == 0), stop=(kx == EK - 1))
                        stg = acp_pool.tile([128, 512], F16, tag="astg")
                        nc.vector.tensor_copy(stg[:], ps[:])
                        nc.sync.dma_start(
                            a2_d[c][:, tb * 512:(tb + 1) * 512], stg[:])

            # ---------- Picard sweeps ----------
            with tc.tile_pool(name="aslp", bufs=4) as aslp, \
                 tc.tile_pool(name="gps", bufs=3, space="PSUM") as gps, \
                 tc.tile_pool(name="sgp", bufs=2) as sgp, \
                 tc.tile_pool(name="wkp", bufs=2) as wkp, \
                 tc.tile_pool(name="csp", bufs=2) as csp:
                for s in range(NSWEEP):
                    cur, nxt = Hp[s % 2], Hp[(s + 1) % 2]
                    last = (s == NSWEEP - 1)
                    cs_prev = None
                    for tb in range(NTB):
                        t0 = tb * TB
                        # S tiles: [grp][cell] -> [128, KC*TB]
                        S = [[sgp.tile([128, KC * TB], F16,
                                       tag=f"S{g}{ce}", name=f"S{g}{ce}")
                              for ce in range(2)] for g in range(4)]
                        # ---- gates: psum = A + W*hprev; sigma/tanh ----
                        # one psum + one activation per (gate-group, cell):
                        # 4 chunk-columns each, A loaded in one strided DMA.
                        for gc in range(8):
                            grp, cell = gc // 2, gc % 2
                            c0 = grp * 8 + cell * 4
                            hoff = 0 if cell == 0 else 4
                            asl = aslp.tile([128, 4 * TB], F16, tag="asl")
                            src = AP(a2_d, c0 * (128 * T) + t0,
                                     [[T, 128], [128 * T, 4], [1, TB]])
                            nc.sync.dma_start(_v4(asl[:]), src)
                            ps = gps.tile([128, 4 * TB], F32, tag="gps")
                            for chunk in range(4):
                                csl = slice(chunk * TB, (chunk + 1) * TB)
                                nc.tensor.matmul(ps[:, csl], ident[:],
                                                 asl[:, csl],
                                                 start=True, stop=(s == 0))
                                if s > 0:
                                    c = c0 + chunk
                                    for k in range(KC):
                                        nc.tensor.matmul(
                                            ps[:, csl],
                                            wsb[:, (c * KC + k) * 128:
                                                (c * KC + k + 1) * 128],
                                            cur[:, (hoff + k) * TP1 + t0:
                                                (hoff + k) * TP1 + t0 + TB],
                                            start=False, stop=(k == KC - 1))
                            nc.scalar.activation(
                                S[grp][cell][:], ps[:],
                                AF.Tanh if grp == 2 else AF.Sigmoid)
                        Si_e, Si_c = S[0]
                        Sf_e, Sf_c = S[1]
                        Tg_e, Tg_c = S[2]
                        So_e, So_c = S[3]
                        # ---- DVE block ----
                        wt = lambda tg: wkp.tile([128, KC * TB], F16,
                                                 tag=tg, name=tg)
                        mk4 = _strided(mTs[:], t0, 0, 4, TB)
                        P_e, P_c = wt("P_e"), wt("P_c")
                        t1, t2, t3, t4 = wt("t1"), wt("t2"), wt("t3"), wt("t4")
                        selE, selC = wt("selE"), wt("selC")
                        tsE, tsC = wt("tsE"), wt("tsC")
                        nc.vector.tensor_tensor(P_e[:], Si_e[:], Tg_e[:],
                                                OP.mult)
                        nc.vector.tensor_tensor(P_c[:], Si_c[:], Tg_c[:],
                                                OP.mult)
                        # t1 = Pe1 = P_e*m ; t2 = Pe0 = P_e - Pe1
                        nc.vector.tensor_tensor(_v4(t1[:]), _v4(P_e[:]),
                                                mk4, OP.mult)
                        nc.vector.tensor_tensor(t2[:], P_e[:], t1[:],
                                                OP.subtract)
                        # t3 = Pc1 ; t4 = Pc0
                        nc.vector.tensor_tensor(_v4(t3[:]), _v4(P_c[:]),
                                                mk4, OP.mult)
                        nc.vector.tensor_tensor(t4[:], P_c[:], t3[:],
                                                OP.subtract)
                        # FF -> Tg_e (tanh-g free after P)
                        nc.vector.tensor_tensor(Tg_e[:], Sf_e[:], Sf_c[:],
                                                OP.mult)
                        # U = f_c*Pe1 + Pc1 + f_e*Pc0 + Pe0 -> t1
                        nc.vector.tensor_tensor(t1[:], Sf_c[:], t1[:],
                                                OP.mult)
                        nc.vector.tensor_tensor(t4[:], Sf_e[:], t4[:],
                                                OP.mult)
                        nc.vector.tensor_tensor(t1[:], t1[:], t4[:], OP.add)
                        nc.vector.tensor_tensor(t2[:], t2[:], t3[:], OP.add)
                        nc.vector.tensor_tensor(t1[:], t1[:], t2[:], OP.add)
                        # ---- c scan ----
                        cS = csp.tile([128, KC * CS], F16, tag="cS",
                                      name="cS")
                        if tb == 0:
                            nc.gpsimd.memset(
                                _strided(cS[:], 0, CS, 4, 1), 0.0)
                        else:
                            nc.vector.tensor_copy(
                                _strided(cS[:], 0, CS, 4, 1),
                                _strided(cs_prev[:], TB, CS, 4, 1))
                        for j in range(KC):
                            ini = 0.0 if tb == 0 else \
                                cs_prev[:, j * CS + TB:j * CS + TB + 1]
                            nc.vector.tensor_tensor_scan(
                                cS[:, j * CS + 1:j * CS + 1 + TB],
                                Tg_e[:, j * TB:(j + 1) * TB],
                                t1[:, j * TB:(j + 1) * TB],
                                ini, OP.mult, OP.add)
                        cprev = _strided(cS[:], 0, CS, 4, TB)
                        c4 = _strided(cS[:], 1, CS, 4, TB)
                        # X_e -> t2 ; X_c -> t3
                        nc.vector.tensor_tensor(_v4(t2[:]), _v4(Sf_e[:]),
                                                cprev, OP.mult)
                        nc.vector.tensor_tensor(t2[:], t2[:], P_e[:], OP.add)
                        nc.vector.tensor_tensor(_v4(t3[:]), _v4(Sf_c[:]),
                                                cprev, OP.mult)
                        nc.vector.tensor_tensor(t3[:], t3[:], P_c[:], OP.add)
                        # SEL_e = c + m*(X_e - c)
                        nc.vector.tensor_tensor(_v4(t4[:]), _v4(t2[:]),
                                                c4, OP.subtract)
                        nc.vector.tensor_tensor(_v4(t4[:]), _v4(t4[:]),
                                                mk4, OP.mult)
                        nc.vector.tensor_tensor(_v4(selE[:]), _v4(t4[:]),
                                                c4, OP.add)
                        # SEL_c = X_c - m*(X_c - c)
                        nc.vector.tensor_tensor(_v4(t4[:]), _v4(t3[:]),
                                                c4, OP.subtract)
                        nc.vector.tensor_tensor(_v4(t4[:]), _v4(t4[:]),
                                                mk4, OP.mult)
                        nc.vector.tensor_tensor(selC[:], t3[:], t4[:],
                                                OP.subtract)
                        nc.scalar.activation(tsE[:], selE[:], AF.Tanh)
                        nc.scalar.activation(tsC[:], selC[:], AF.Tanh)
                        # h -> Hp[nxt] shifted by +1 col
                        hpE = _strided(nxt[:], 0 * TP1 + t0 + 1, TP1, 4, TB)
                        hpC = _strided(nxt[:], 4 * TP1 + t0 + 1, TP1, 4, TB)
                        nc.vector.tensor_tensor(hpE, _v4(So_e[:]),
                                                _v4(tsE[:]), OP.mult)
                        nc.vector.tensor_tensor(hpC, _v4(So_c[:]),
                                                _v4(tsC[:]), OP.mult)
                        if last:
                            ovT = _strided(outsT[:], t0, T, 4, TB)
                            nc.vector.tensor_tensor(ovT, hpE, hpC, OP.add)
                        cs_prev = cS

        # ---------- fc1 + fc2, fused per t-block (hmid stays in SBUF) ----
        MGROUPS = [(0, 4), (4, 4), (8, 2)]
        with tc.tile_pool(name="f1w", bufs=1) as f1w, \
             tc.tile_pool(name="fhm", bufs=1) as fhm, \
             tc.tile_pool(name="f1ps", bufs=2, space="PSUM") as f1ps, \
             tc.tile_pool(name="w2p", bufs=6) as w2p, \
             tc.tile_pool(name="f2ps", bufs=5, space="PSUM") as f2ps, \
             tc.tile_pool(name="f2st", bufs=4) as f2st:
            w1sb = f1w.tile([128, 4 * VP], F16, tag="w1sb")
            for k in range(4):
                nc.sync.dma_start(w1sb[:, k * VP:(k + 1) * VP],
                                  w1t_d[k * 128:(k + 1) * 128, :])
            for nb in range(4):
                hmB = fhm.tile([128, MT1 * 512], F16, tag="hmB")
                for m in range(MT1):
                    ps = f1ps.tile([128, 512], F32, tag="f1p")
                    for k in range(4):
                        nc.tensor.matmul(
                            ps[:],
                            w1sb[:, k * VP + m * 128: k * VP + m * 128 + 128],
                            outsT[:, k * T + nb * 512: k * T + nb * 512 + 512],
                            start=(k == 0), stop=(k == 3))
                    nc.scalar.activation(hmB[:, m * 512:(m + 1) * 512],
                                         ps[:], AF.Relu, bias=b1c[:, m:m + 1])
                for (m0, mw) in MGROUPS:
                    pss = [f2ps.tile([128, 512], F32, tag="f2p", name="f2p")
                           for _ in range(mw)]
                    for k2 in range(MT1):
                        w2c = w2p.tile([128, 512], F16, tag="w2c")
                        nc.sync.dma_start(
                            w2c[:, 0:mw * 128],
                            w2t_d[k2 * 128:(k2 + 1) * 128,
                                  m0 * 128: m0 * 128 + mw * 128])
                        for mi in range(mw):
                            nc.tensor.matmul(
                                pss[mi][:], w2c[:, mi * 128:(mi + 1) * 128],
                                hmB[:, k2 * 512:(k2 + 1) * 512],
                                start=(k2 == 0), stop=(k2 == MT1 - 1))
                    for mi in range(mw):
                        m = m0 + mi
                        ost = f2st.tile([128, 512], F32, tag="f2o")
                        nc.scalar.activation(ost[:], pss[mi][:], AF.Identity,
                                             bias=b2c[:, m:m + 1])
                        nc.sync.dma_start(
                            outT_d[m * 128:(m + 1) * 128,
                                   nb * 512:(nb + 1) * 512], ost[:])

    nc.compile()
    return nc


# ---------------- host side ----------------

_NC_CACHE = {}


def _get_program():
    if "nc" not in _NC_CACHE:
        _NC_CACHE["nc"] = build_program()
    return _NC_CACHE["nc"]


def _build_big(Wen, Wcn):
    """Stack two cells' torch-gate-order rows [i,f,g,o] into combined
    [i_en, i_cn, f_en, f_cn, g_en, g_cn, o_en, o_cn] order."""
    blocks = []
    for gi in range(4):
        blocks.append(Wen[gi * H:(gi + 1) * H])
        blocks.append(Wcn[gi * H:(gi + 1) * H])
    return np.concatenate(blocks, axis=0)


def _pack_lhsT(bigw, nk):
    """[4096, nk*128] -> [128, 32*nk*128] with tile (c,k) at col
    (c*nk+k)*128 + m, element [p] = bigw[c*128+m, k*128+p]."""
    arr = bigw.reshape(NCOLS, 128, nk, 128)           # [c, m, k, p]
    return np.ascontiguousarray(arr.transpose(3, 0, 2, 1)
                                ).reshape(128, NCOLS * nk * 128)


def host_prep(inputs):
    tok = np.asarray(inputs["token_ids"]).astype(np.int64)
    msk = np.asarray(inputs["mask"]).astype(np.float32)
    emb = np.asarray(inputs["emb"], dtype=np.float32)
    f32 = lambda n: np.asarray(inputs[n], dtype=np.float32)
    Wih_en, Whh_en = f32("Wih_en"), f32("Whh_en")
    bih_en, bhh_en = f32("bih_en"), f32("bhh_en")
    Wih_cn, Whh_cn = f32("Wih_cn"), f32("Whh_cn")
    bih_cn, bhh_cn = f32("bih_cn"), f32("bhh_cn")
    fc1_W, fc1_b = f32("fc1_W"), f32("fc1_b")
    fc2_W, fc2_b = f32("fc2_W"), f32("fc2_b")

    # --- recurrent weights ---
    bigwhh = _build_big(Whh_en, Whh_cn)               # [4096, 512]
    wsb = _pack_lhsT(bigwhh, KC).astype(np.float16)

    # --- A-stage weights: [Wih | b | 0] augmented to K=384 ---
    def aug(Wih, b):
        return np.concatenate(
            [Wih, b[:, None],
             np.zeros((4 * H, EK * 128 - E - 1), np.float32)], axis=1)
    ae = aug(Wih_en, bih_en + bhh_en)                 # [2048, 384]
    ac = aug(Wih_cn, bih_cn + bhh_cn)
    bigwih = _build_big(ae, ac)                       # [4096, 384]
    wih = _pack_lhsT(bigwih, EK).astype(np.float16)

    # --- X augmented, mask-folded, transposed ---
    X = emb[tok]                                      # [T, E]
    ones = np.ones((T, 1), np.float32)
    zpad = np.zeros((T, EK * 128 - E - 1), np.float32)
    xa_en = np.concatenate([X * msk[:, None], ones, zpad], axis=1)
    xa_cn = np.concatenate([X * (1.0 - msk)[:, None], ones, zpad], axis=1)
    xte = np.ascontiguousarray(
        xa_en.reshape(T, EK, 128).transpose(2, 1, 0)).reshape(128, EK * T)
    xtc = np.ascontiguousarray(
        xa_cn.reshape(T, EK, 128).transpose(2, 1, 0)).reshape(128, EK * T)

    mT = np.ascontiguousarray(
        np.broadcast_to(msk[None, :], (128, T))).astype(np.float16)
    ident = np.eye(128, dtype=np.float16)

    # --- fc1 ---
    w1p = np.zeros((VP, H), np.float32)
    w1p[:V] = fc1_W
    w1t = np.ascontiguousarray(w1p.T).astype(np.float16)   # [512, VP]
    b1p = np.zeros((VP,), np.float32)
    b1p[:V] = fc1_b
    b1c = np.ascontiguousarray(b1p.reshape(MT1, 128).T)    # [128, MT1]

    # --- fc2 shards ---
    shard_w, shard_b = [], []
    for s in range(NCORES):
        w2p_ = np.zeros((VSP, VP), np.float32)
        w2p_[:VSH, :V] = fc2_W[s * VSH:(s + 1) * VSH]
        shard_w.append(np.ascontiguousarray(w2p_.T).astype(np.float16))
        b2p = np.zeros((VSP,), np.float32)
        b2p[:VSH] = fc2_b[s * VSH:(s + 1) * VSH]
        shard_b.append(np.ascontiguousarray(b2p.reshape(MT2, 128).T))

    common = dict(wsb=wsb, wih=wih, xte=xte.astype(np.float16),
                  xtc=xtc.astype(np.float16), mT=mT, ident=ident,
                  w1t=w1t, b1c=b1c)
    in_maps = []
    for s in range(NCORES):
        m = dict(common)
        m["w2t"] = shard_w[s]
        m["b2c"] = shard_b[s]
        in_maps.append(m)
    return in_maps


LAST_RESULT = None


def _install_ntff_shim():
    """The agent image lacks antenv.axon_hooks; register the ctypes NTFF
    profiling hook manually so trace=True works."""
    import sys, types
    if "antenv.axon_hooks" in sys.modules:
        return
    import antenv
    mod = types.ModuleType("antenv.axon_hooks")
    _h = [None]
    mod.set_axon_ntff_profile_hook = lambda h: _h.__setitem__(0, h)
    mod.get_axon_ntff_profile_hook = lambda: _h[0]
    sys.modules["antenv.axon_hooks"] = mod
    antenv.axon_hooks = mod
    from trn_agent_boot.trn_boot import _ntff_profile_via_ctypes
    mod.set_axon_ntff_profile_hook(
        _ntff_profile_via_ctypes("/opt/axon/libaxon_pjrt.so"))


def kernel(**inputs):
    global LAST_RESULT
    trace = bool(os.environ.get("DUALLSTM_TRACE"))
    if trace:
        _install_ntff_shim()
    nc = _get_program()
    in_maps = host_prep(inputs)
    res = run_bass_kernel_spmd(nc, in_maps, core_ids=list(range(NCORES)),
                               trace=trace)
    LAST_RESULT = res
    out = np.empty((T, V), np.float32)
    for s in range(NCORES):
        out[:, s * VSH:(s + 1) * VSH] = res.results[s]["outT"][:VSH].T
    return out
